# revision 32
# baseline (speedup 1.0000x reference)
"""DSAttention (de-stationary causal attention) Trainium2 Bass kernel.

Problem: B=4, L=S=2048, H=8, E=D=64, fp32.
  scores = (Q K^T) * tau_b + delta_b[s]      [B,H,L,S]
  A      = softmax(0.125 * scores) causal-masked
  out    = A V                               [B,L,H,D]

Sharding: the 32 (b,h) pairs are split 4-per-core across 8 NeuronCores
(data + head parallel). Each core runs an identical SPMD Bass program on
its own 4 pairs; no collectives are needed.

Per-core algorithm (two PAIRED windows, one per pair-pair):
  - Scores are computed TRANSPOSED, S^T[s, l], one 128-row s-chunk at a
    time. The two pairs of a pair-pair share ONE [128, L] k^T stationary
    tile (rows 0:64 = pair A, 64:128 = pair B); each pair's q^T moving
    tile is [128, L] with the OTHER pair's partition half ZEROED, so a
    full K=128 matmul computes that pair's scores exactly. K=128 matters
    beyond sharing one ldweights: the PE_HAM clock governor does NOT
    count K<=64 matmuls as activity (verified on HW - K=64/K=1 streams
    never lift the PE out of its 1.2GHz cold state), while this layout
    latches 2.4GHz right at the first QK burst and holds it.
  - q/k/v are loaded via gpsimd-initiated CASTING DMAs (fp32 DRAM ->
    bf16 SBUF in flight). The single SWDGE ring sustains ~130GB/s and
    carries ~6MB, so it is nearly co-critical: pair-pair 0 loads the
    tail half first (its chunks run high-c-first, MIX0), and the next
    window's q/k prefetch is emitted early in the current window.
    (Splitting the feed onto the sync HWDGE ring was tried twice and is
    SLOWER - that ring is no faster and the extra casts/staging stall.)
  - exp of the scores is split between two engines by a greedy
    makespan assignment with per-engine time models (ACT (w+420)/1.2,
    DVE (w*1.042+276)*DVE_LOAD) and an alternation preference (the
    3-tile psum ring buffers only ~1 piece, so same-engine streaks
    stall the other engine): ACT runs exact exp(scale*qk + bias) with
    fused per-partition scale=0.125*tau / bias=0.125*delta^T columns;
    DVE runs a Schraudolph fast-exp producing bf16 BITS via one
    tensor_scalar: int16(qk*c1 + c2), bitcast to bf16 (~2% rms error on
    its share). GpSimd cannot read PSUM and DMA has no PSUM route, so
    ACT+DVE are the ONLY possible score consumers - their combined
    throughput is the kernel's floor.
  - The causal mask only affects the diagonal 128x128 block of each
    chunk; it is applied AFTER exp as a triangle zeroing via
    gpsimd.affine_select (keeps the hot ACT/DVE engines free).
  - AV uses A^T chunks (bf16) as the 128x128 stationary operand and an
    augmented moving operand [v | 1] of 65 columns, so column 64 of the
    PSUM accumulator is the softmax denominator (ldweights pipelines
    into the background weight buffer; AV sustains ~33ns/step warm).
    Output tiles come out in natural [l, d] layout; DVE computes
    reciprocal + scale, sync DMAs store.
  - SCHEDULE: window 0 = QK(pp0) exp-paced; window 1 = QK(pp1) with
    pair-pair 0's deferred AV pumped between score pieces in small
    budget-paced units (coarse AV bursts starve the exp engines);
    window-1's own AV fires inline as chunks complete, with a pump
    floor late in the window so the no-exp-overlap tail stays short.
    K=128 garbage-weight filler matmuls ramp the PE clock during the
    input DMA dead time (the fillers read a raw uninitialized SBUF
    tensor so they depend on nothing after the engine barrier).
"""

import numpy as np

try:
    import concourse.bass as bass
except ImportError:  # toolchain not on default path
    import sys

    sys.path.insert(0, "/opt/trn_rl_repo")
    import concourse.bass as bass

import concourse.mybir as mybir
import concourse.tile as tile
from concourse import bacc
from concourse.bass_utils import run_bass_kernel_spmd

B, L, H, E, D = 4, 2048, 8, 64, 64
NCORES = 8
PAIRS = B * H            # 32 (b,h) pairs
PPC = PAIRS // NCORES    # 4 pairs per core
NT = L // 128            # 16 s-chunks / l-tiles per pair
MMW = 512                # max moving width per fp32-out matmul (1 PSUM bank)

F32 = mybir.dt.float32
BF16 = mybir.dt.bfloat16

import os as _os

CFG = {
    "PIECE": int(_os.environ.get("DSATT_PIECE", "1024")),
    "PS_BUFS": int(_os.environ.get("DSATT_PS_BUFS", "3")),
    "JBATCH": int(_os.environ.get("DSATT_JBATCH", "7")),
    "QSPLIT": int(_os.environ.get("DSATT_QSPLIT", "2")),  # first pair-pair
    "QSPLIT2": int(_os.environ.get("DSATT_QSPLIT2", "2")),  # later pair-pairs
    "TAILSPLIT": int(_os.environ.get("DSATT_TAILSPLIT", "1")),
    "BCAST_NORM": int(_os.environ.get("DSATT_BCAST_NORM", "1")),
    # legalize multi-wait matmuls via event semaphores instead of bacc's
    # move_matmul_waits_to_ldweights pass (HW-validated variant)
    "SELF_LOAD": int(_os.environ.get("DSATT_SELF_LOAD", "1")),
    # fraction of exp work offloaded from ACT to DVE / GpSimd using the
    # Schraudolph bf16-bits fast-exp (int16(x*c1+c2) bitcast to bf16)
    # NOTE: GpSimd cannot read PSUM, so only ACT and DVE can consume QK
    # scores; EXP_POOL must stay 0 unless scores are staged to SBUF first.
    "EXP_DVE": float(_os.environ.get("DSATT_EXP_DVE", "0.44")),
    "EXP_POOL": float(_os.environ.get("DSATT_EXP_POOL", "0.0")),
    # defer each pair's AV into the next pair's QK window (0 = fire AV
    # batches inline as soon as their chunks are ready, as before)
    "AV_DEFER": int(_os.environ.get("DSATT_AV_DEFER", "1")),
    # deferred-AV pump ratio: AV cycles emitted per QK cycle in the next
    # pair's window
    "AV_RATIO": float(_os.environ.get("DSATT_AV_RATIO", "1.0")),
    # filler matmuls at kernel start to ramp the PE clock during input DMA
    "WARM": int(_os.environ.get("DSATT_WARM", "16")),
    # window-0 pacing: filler matmul columns emitted after each pair-0 QK
    # piece (pair 0 has no deferred AV to interleave, so raw QK outruns
    # the exp consumers, stalls, and resets the PE pstate ramp)
    "W0PACE": int(_os.environ.get("DSATT_W0PACE", "0")),
    # pair-0 mixed chunk order: this many high chunks first, then 0..N.
    # 16 = pure high-first (verified schedule; MIX0=4 raised the error to
    # 3.95e-2 by clustering fast-exp pieces onto low-l rows)
    "MIX0": int(_os.environ.get("DSATT_MIX0", "8")),
    # order-independent exp engine assignment (enables safe reordering)
    "EXP_HASH": int(_os.environ.get("DSATT_EXP_HASH", "0")),
    "WARM_COLS": int(_os.environ.get("DSATT_WARM_COLS", "512")),
    # run QK matmuls in float32r straight from the fp32 q/k tiles (no
    # casts). BROKEN on HW: walrus fp32r weights via split ldweights give
    # wrong results (rel err ~1). Keep 0.
    "QK_F32R": int(_os.environ.get("DSATT_QK_F32R", "0")),
    # move q/k bf16 casts for pair-pairs > 0 to GpSimd (SBUF->SBUF ok)
    "CAST_POOL": int(_os.environ.get("DSATT_CAST_POOL", "0")),
    # move v bf16 casts to GpSimd
    "VCAST_POOL": int(_os.environ.get("DSATT_VCAST_POOL", "0")),
    # fp32->bf16 casting DMAs (gpsimd-initiated) for q/k and v loads:
    # 2 = all pair-pairs, 1 = pair-pairs > 0 only, 0 = DVE casts.
    "QKDMA_CAST": int(_os.environ.get("DSATT_QKDMA_CAST", "2")),
    "VDMA_CAST": int(_os.environ.get("DSATT_VDMA_CAST", "1")),
    # split the input feed across DMA rings: qB + v via sync(HWDGE) fp32
    # + gpsimd SBUF casts instead of gpsimd casting DMAs
    "SPLIT_FEED": int(_os.environ.get("DSATT_SPLIT_FEED", "0")),
}

# fast-exp: bf16 bits of exp(v) ~= round(v * A16 + B16) taken as int16
FE_A16 = 128.0 / float(np.log(2.0))  # 184.6650
FE_B16 = float(_os.environ.get("DSATT_FE_B16", "16250.4"))  # 16256 - C*


def _compile_no_ldw_split(nc):
    """bacc.Bacc.compile() minus move_matmul_waits_to_ldweights: keeps
    matmuls self-loading; generate_event_semaphores legalizes waits."""
    from concourse import inst_simplify

    nc.insert_bir_kernel_barrier_sem_inc()
    nc.generate_event_semaphores()
    nc.remove_dead_instructions_after_branch()
    nc.validate_blocks()
    nc.dce_regs()
    nc.thread_jumps()
    nc.remove_dead_blocks()
    nc.remove_dead_allocations()
    nc.verify_switch_hints()
    nc.alloc_regs()
    inst_simplify.simplify(nc)
    nc.fuse_regops()
    nc.fuse_blocks()
    nc.replace_nops_with_events()
    for engine in nc.engines:
        nc.fuse_nops(engine)
    nc.remove_dead_nops()
    nc.remove_dangling_data()
    nc.generate_event_semaphores()
    nc.insert_library_loads()
    nc.insert_act_table_loads()
    nc.insert_hostgen_rebases()
    nc.codegen_inst_isa_subclasses()


def _emit(tc, qt, kt, v, tau4, deltat, out):
    nc = tc.nc
    Exp = mybir.ActivationFunctionType.Exp
    from contextlib import ExitStack

    PIECE = CFG["PIECE"]
    JBATCH = CFG["JBATCH"]
    QSPLIT = CFG["QSPLIT"]

    ctx = ExitStack()
    const = ctx.enter_context(tc.tile_pool(name="const", bufs=1))
    qk_pool = ctx.enter_context(tc.tile_pool(name="qk", bufs=2))
    qkb_pool = ctx.enter_context(tc.tile_pool(name="qkb", bufs=2))
    v_pool = ctx.enter_context(tc.tile_pool(name="vp", bufs=3))
    vb_pool = ctx.enter_context(tc.tile_pool(name="vbp", bufs=3))
    at_pool = ctx.enter_context(tc.tile_pool(name="atp", bufs=2))
    ob_pool = ctx.enter_context(tc.tile_pool(name="obp", bufs=3))
    ps_pool = ctx.enter_context(tc.tile_pool(name="psp", bufs=CFG["PS_BUFS"], space="PSUM"))
    po_pool = ctx.enter_context(tc.tile_pool(name="pop", bufs=2, space="PSUM"))

    # ---- PE clock pre-warm: filler matmuls while input DMA is in flight.
    # The TRN2 tensor engine runs at the 1.2GHz mid pstate until it has been
    # continuously busy for ~3us; these ramp it so real QK work starts fast.
    # The filler operand is a RAW (non-tile) SBUF tensor read uninitialized:
    # any init (memset/DMA) would chain the first filler behind another
    # engine's startup, delaying the HAM warm ramp. Garbage values are
    # harmless - the filler PSUM banks are fully overwritten (start=True)
    # by every real matmul before being read.
    # K=128 full-array fillers: low-utilization matmuls (K=1) do NOT
    # register as busy to the PE_HAM activity monitor (verified on HW: a
    # 4.5us continuous K=1 filler block left the clock at 1.2GHz), so the
    # warm-up must exercise all 128 rows.
    cw_h = ctx.enter_context(nc.sbuf_tensor("warm_cw", [128, 512], BF16))
    cw = cw_h.ap()
    if CFG["WARM"]:
        wc = min(CFG["WARM_COLS"], CFG["PIECE"])
        pw = ps_pool.tile([128, CFG["PIECE"]], F32, tag="ps", name="warm")
        for _ in range(CFG["WARM"]):
            nc.tensor.matmul(pw[:, 0:wc], lhsT=cw[:, 0:128], rhs=cw[:, 0:wc], start=True, stop=True)
    # preload the ACT exp spline tables (~1.3us) off the critical path: the
    # compiler inserts ACT_TABLE_LOAD before the first ACTIVATE in program
    # order, which would otherwise sit right before the first real exp
    warm_act = const.tile([1, 8], BF16)
    nc.scalar.activation(warm_act[:], cw[0:1, 0:8], Exp)

    # ---- q/k loads for a pair-pair ---------------------------------------
    # k is one SHARED [128, L] stationary tile (rows 0:64 = pair A's k^T,
    # 64:128 = pair B's). Each pair's q is a SEPARATE [128, L] moving tile
    # with the OTHER pair's partition half zeroed, so a full K=128 matmul
    # against the shared k computes that pair's scores exactly (the other
    # half contributes 0). K=128 matmuls register as PE_HAM activity
    # (K<=64 ones do NOT - verified on HW), keeping the 2.4GHz clock, and
    # one ldweights serves both pairs.
    def emit_qk_loads(pp):
        kb = qkb_pool.tile([128, L], BF16, tag="kb")
        qbA = qkb_pool.tile([128, L], BF16, tag="qa")
        qbB = qkb_pool.tile([128, L], BF16, tag="qb")
        # zero the unused partition half of each q tile (DVE is idle at
        # startup; prefetched window-1 tiles are zeroed on gpsimd)
        mset = nc.vector.memset if pp == 0 else nc.gpsimd.memset
        mset(qbA[64:128, :], 0.0)
        mset(qbB[0:64, :], 0.0)
        kdram = kt[2 * pp : 2 * pp + 2].rearrange("a e l -> (a e) l")
        nsplit = QSPLIT if pp == 0 else CFG["QSPLIT2"]
        w = L // nsplit
        # pair-pair 0 runs its chunks high-c-first (they need only the tail
        # columns of q/k), so load pieces tail-first to start PE early
        order = range(nsplit - 1, -1, -1) if pp == 0 else range(nsplit)
        for i in order:
            s = slice(w * i, w * (i + 1))
            nc.gpsimd.dma_start(kb[:, s], kdram[:, s])
            nc.gpsimd.dma_start(qbA[0:64, s], qt[2 * pp][:, s])
            if CFG["SPLIT_FEED"]:
                qf = qk_pool.tile([128, w], F32, tag="qf", name=f"qf_{pp}_{i}")
                nc.sync.dma_start(qf[64:128, :], qt[2 * pp + 1][:, s])
                nc.gpsimd.tensor_copy(qbB[64:128, s], qf[64:128, :])
            else:
                nc.gpsimd.dma_start(qbB[64:128, s], qt[2 * pp + 1][:, s])
        return kb, qbA, qbB

    # setup DMAs go first on the sync queue so the ptau matmul (PE queue,
    # right after the warm fillers) isn't blocked behind bulk q/k loads
    c0125 = const.tile([1, 128], F32)
    nc.vector.memset(c0125[:], 0.125)
    tau_sb = const.tile([1, PPC], F32)
    nc.sync.dma_start(tau_sb[:], tau4[:])
    dts = const.tile([128, PPC * NT], F32)
    nc.sync.dma_start(dts[:], deltat[:])

    # pair-pair 0's loads go next so the first QK matmul isn't queued
    # behind the tau/delta setup chain on DVE
    _cur_qk = emit_qk_loads(0)

    # ---- one-time setup -------------------------------------------------
    # Broadcast 0.125*tau[p] to all 128 partitions via a K=1 matmul.
    ptau = po_pool.tile([128, JBATCH * (D + 1)], F32, tag="po", name="ptau")
    nc.tensor.matmul(ptau[:, 0:PPC], lhsT=c0125[:], rhs=tau_sb[:], start=True, stop=True)
    tau_cols = const.tile([128, PPC], F32)
    nc.vector.tensor_copy(tau_cols[:], ptau[:, 0:PPC])

    # bias columns: 0.125 * delta^T  ([128, PPC*NT], column p*NT+c)
    bias_all = const.tile([128, PPC * NT], F32)
    nc.vector.tensor_scalar_mul(bias_all[:], dts[:], 0.125)

    # fast-exp scale/bias columns: int16 bf16-bits = qk*(A16*0.125*tau) +
    # (A16*0.125*delta + B16)
    tsc1 = const.tile([128, PPC], F32)
    nc.vector.tensor_scalar_mul(tsc1[:], tau_cols[:], FE_A16)
    fb = const.tile([128, PPC * NT], F32)
    nc.vector.tensor_scalar(
        fb[:], bias_all[:], FE_A16, FE_B16,
        mybir.AluOpType.mult, mybir.AluOpType.add,
    )

    # greedy makespan assignment of exp pieces using per-engine TIME
    # models (ACT: (w+420)/1.2 ns; DVE: w*1.042+276 ns, inflated by
    # DVE_LOAD for its reciprocal/normalize side work), with a mild
    # preference to alternate engines (the 3-deep psum ring can only
    # buffer ~1 piece, so same-engine streaks stall the other engine)
    exp_done = {"act": 0.0, "dve": 0.0}
    _last_eng = [None]
    _dve_load = float(_os.environ.get("DSATT_DVE_LOAD", "1.12"))

    def next_exp_engine(w, pair, c, pidx):
        t = {"act": (w + 420) / 1.2, "dve": (w * 1.042 + 276) * _dve_load}
        cand = sorted(("act", "dve"), key=lambda e: exp_done[e] + t[e])
        e = cand[0]
        if e == _last_eng[0] and (
            exp_done[cand[1]] + t[cand[1]] - (exp_done[e] + t[e]) < 500
        ):
            e = cand[1]
        exp_done[e] += t[e]
        _last_eng[0] = e
        return e

    I16 = mybir.dt.int16

    def emit_exp(atc_slice, pst_slice, pair, c, w, pidx):
        eng = next_exp_engine(w, pair, c, pidx)
        if eng == "act":
            nc.scalar.activation(
                atc_slice, pst_slice, Exp,
                bias=bias_all[:, NT * pair + c : NT * pair + c + 1],
                scale=tau_cols[:, pair : pair + 1],
            )
        else:
            e = nc.vector if eng == "dve" else nc.gpsimd
            e.tensor_scalar(
                atc_slice.bitcast(I16), pst_slice,
                tsc1[:, pair : pair + 1],
                fb[:, NT * pair + c : NT * pair + c + 1],
                mybir.AluOpType.mult, mybir.AluOpType.add,
            )

    # ---- deferred-AV queue: each pair's AV matmuls are emitted during
    # the NEXT pair's QK window, interleaved ~1:1 in PE cycles. The PE
    # then alternates QK pieces (which need exp consumers) with AV steps
    # (which don't), halving the burst demand on ACT/DVE and avoiding
    # the PSUM-full stalls that knock the PE down to its 1.2GHz pstate.
    av_queue = []  # FIFO of (est_pe_cycles, emit_thunk, is_batch_start)

    def pump_av(budget):
        while av_queue and budget > 0:
            cyc, fn, _ = av_queue.pop(0)
            fn()
            budget -= cyc

    def build_av_units(pair, ats, vb3, j0, nj):
        units = []
        box = {}

        def mmj(jl):
            if jl == 0:
                box["pot"] = po_pool.tile(
                    [128, JBATCH * (D + 1)], F32, tag="po", name=f"po_{pair}_{j0}"
                )
            pot = box["pot"]
            j = j0 + jl
            for cc in range(j + 1):
                nc.tensor.matmul(
                    pot[:, 65 * jl : 65 * jl + 65],
                    lhsT=ats[cc][:, 128 * (j - cc) : 128 * (j - cc) + 128],
                    rhs=vb3[:, cc, :],
                    start=(cc == 0),
                    stop=(cc == j),
                )

        for jl in range(nj):
            units.append(((j0 + jl + 1) * 128, (lambda jl=jl: mmj(jl)), jl == 0))

        def fin():
            pot3 = box["pot"].rearrange("q (jl x) -> q jl x", x=D + 1)
            rec = ob_pool.tile([128, JBATCH], F32, tag="rec", name=f"rec_{pair}_{j0}")
            nc.vector.reciprocal(rec[:, 0:nj], pot3[:, 0:nj, D])
            ob = ob_pool.tile([128, JBATCH * D], F32, tag="ob", name=f"ob_{pair}_{j0}")
            ob3 = ob.rearrange("q (jl d) -> q jl d", d=D)
            if CFG.get("BCAST_NORM", 1):
                in0b, in1b = bass.broadcast_tensor_aps(
                    pot3[:, 0:nj, 0:D], rec[:, 0:nj].unsqueeze(2)
                )
                nc.vector.tensor_tensor(
                    out=ob3[:, 0:nj, :], in0=in0b, in1=in1b,
                    op=mybir.AluOpType.mult,
                )
            else:
                for jl in range(nj):
                    nc.vector.tensor_scalar_mul(
                        ob[:, D * jl : D * jl + D],
                        pot3[:, jl, 0:D],
                        rec[:, jl : jl + 1],
                    )
            nc.sync.dma_start(
                out[pair, 128 * j0 : 128 * (j0 + nj), :].rearrange(
                    "(jl q) d -> q jl d", q=128
                ),
                ob.rearrange("q (jl d) -> q jl d", d=D)[:, 0:nj, :],
            )

        units.append((300, fin, False))
        return units

    # ---- paired windows: the two pairs of a pair-pair (row halves h0 /
    # h64 of the PE array) run their QK matmuls ADJACENTLY, so the two
    # K=64 matmuls execute CONCURRENTLY in disjoint row groups (~2x QK
    # throughput). Window 0 = QK(pp0) [exp-paced; PE has headroom],
    # window 1 = QK(pp1) + deferred AV(pp0); tail = leftover AV(pp1),
    # partially pulled inline as its chunks complete.
    def emit_v_load(pair):
        vb = vb_pool.tile([128, NT * (D + 1)], BF16, tag="vb")
        vb3 = vb.rearrange("q (c x) -> q c x", x=D + 1)
        nc.gpsimd.memset(vb3[:, :, D : D + 1], 1.0)
        if CFG["SPLIT_FEED"]:
            # sync (HWDGE) fp32 load + gpsimd cast: keeps the v traffic
            # off the oversubscribed gpsimd SWDGE DMA ring
            vf = v_pool.tile([128, NT * D], F32, tag="vf")
            nc.sync.dma_start(
                vf.rearrange("q (c d) -> q c d", d=D),
                v[pair].rearrange("(c q) d -> q c d", q=128),
            )
            nc.gpsimd.tensor_copy(
                vb3[:, :, 0:D], vf.rearrange("q (c d) -> q c d", d=D)
            )
        elif CFG["VDMA_CAST"]:
            nc.gpsimd.dma_start(
                vb3[:, :, 0:D],
                v[pair].rearrange("(c q) d -> q c d", q=128),
            )
        else:
            vf = v_pool.tile([128, NT * D], F32, tag="vf")
            nc.sync.dma_start(
                vf.rearrange("q (c d) -> q c d", d=D),
                v[pair].rearrange("(c q) d -> q c d", q=128),
            )
            nc.vector.tensor_copy(
                vb3[:, :, 0:D], vf.rearrange("q (c d) -> q c d", d=D)
            )
        return vb3

    def make_batches(pair):
        batches = []
        j0_ = 0
        while j0_ < NT:
            nj_ = min(JBATCH, NT - j0_)
            if CFG["TAILSPLIT"] and pair >= PPC - 2 and j0_ + nj_ == NT and nj_ > 1:
                batches.append((j0_, nj_ - 1))
                batches.append((j0_ + nj_ - 1, 1))
            else:
                batches.append((j0_, nj_))
            j0_ += nj_
        return batches

    NPP = PPC // 2
    _prefetched_qk = None
    for pp in range(NPP):
        pA, pB = 2 * pp, 2 * pp + 1
        if pp > 0:
            _cur_qk = _prefetched_qk if _prefetched_qk else emit_qk_loads(pp)
        kb, qbA, qbB = _cur_qk
        qtiles = {pA: qbA, pB: qbB}
        vb3s = {pA: emit_v_load(pA), pB: emit_v_load(pB)}
        batches = {pA: make_batches(pA), pB: make_batches(pB)}
        ats = {pA: [None] * NT, pB: [None] * NT}
        done = {pA: set(), pB: set()}
        fired = {pA: set(), pB: set()}
        last_pp = pp == NPP - 1
        ratio = CFG["AV_RATIO"] if CFG["AV_DEFER"] else float(1 << 20)
        if pp == 0:
            # high chunks first (need only tail q/k DMA pieces), then
            # ascending so low chunks complete early for AV readiness
            m0 = CFG["MIX0"]
            chunk_iter = list(range(NT - 1, NT - 1 - m0, -1)) + list(range(NT - m0))
        else:
            chunk_iter = range(NT)
        _next_qk = None
        for ci, c in enumerate(chunk_iter):
            # prefetch the next pair-pair's q/k early in this window so
            # its ~2MB casting DMA overlaps this window's exp-paced work
            if ci == 2 and pp + 1 < NPP:
                _next_qk = emit_qk_loads(pp + 1)
            ext = L - 128 * c
            for p in (pA, pB):
                ats[p][c] = at_pool.tile(
                    [128, ext], BF16, tag=f"at{c}{p % 2}", name=f"at{c}_{p}"
                )
            l0 = 128 * c
            while l0 < L:
                w = min(PIECE, L - l0)
                pst = {
                    p: ps_pool.tile([128, PIECE], F32, tag="ps", name=f"ps_{p}_{c}_{l0}")
                    for p in (pA, pB)
                }
                # full K=128 matmuls against the shared k stationary; the
                # other pair's zeroed q half contributes exact zeros.
                # 512-aligned subs (PSUM bank rule); pieces alternate pairs
                # so the exp deficit scheduler alternates ACT/DVE.
                for p in (pA, pB):
                    for s0 in range(0, w, MMW):
                        sw = min(MMW, w - s0)
                        nc.tensor.matmul(
                            pst[p][:, s0 : s0 + sw],
                            lhsT=kb[:, 128 * c : 128 * (c + 1)],
                            rhs=qtiles[p][:, l0 + s0 : l0 + s0 + sw],
                            start=True,
                            stop=True,
                        )
                off = l0 - 128 * c
                for p in (pA, pB):
                    emit_exp(
                        ats[p][c][:, off : off + w], pst[p][:, 0:w], p, c, w, off // PIECE
                    )
                if not CFG["AV_DEFER"]:
                    pump_av(1 << 30)
                elif ci < NT - 6:
                    pump_av(int(2 * w * ratio))
                else:
                    # late-window: QK pieces shrink while the AV backlog
                    # peaks - pump with an absolute floor so the kernel
                    # tail (AV with no exp overlap) stays short
                    pump_av(max(int(2 * w * 3.0), 8000))
                l0 += w
            for p in (pA, pB):
                # zero the strictly-lower triangle (s > l) of the diag block
                nc.gpsimd.affine_select(
                    out=ats[p][c][:, 0:128],
                    in_=ats[p][c][:, 0:128],
                    compare_op=mybir.AluOpType.is_ge,
                    fill=0.0,
                    base=0,
                    pattern=[[1, 128]],
                    channel_multiplier=-1,
                )
                done[p].add(c)
                # queue ready AV batches inline (their leftovers carry
                # into the next window's pump)
                for j0, nj in batches[p]:
                    need = j0 + nj - 1
                    if (j0, nj) in fired[p] or not all(
                        cc in done[p] for cc in range(need + 1)
                    ):
                        continue
                    fired[p].add((j0, nj))
                    av_queue.extend(build_av_units(p, ats[p], vb3s[p], j0, nj))
        # window end: queue any remaining batches for the next window
        _prefetched_qk = _next_qk
        for p in (pA, pB):
            for j0, nj in batches[p]:
                if (j0, nj) not in fired[p]:
                    fired[p].add((j0, nj))
                    av_queue.extend(build_av_units(p, ats[p], vb3s[p], j0, nj))
    pump_av(1 << 30)
    ctx.close()


_NC_CACHE = {}


def _get_nc():
    if "nc" not in _NC_CACHE:
        nc = bacc.Bacc("TRN2", target_bir_lowering=False, debug=False)
        qt = nc.dram_tensor("qt", [PPC, E, L], F32, kind="ExternalInput")
        kt = nc.dram_tensor("kt", [PPC, E, L], F32, kind="ExternalInput")
        v = nc.dram_tensor("v", [PPC, L, D], F32, kind="ExternalInput")
        tau4 = nc.dram_tensor("tau4", [1, PPC], F32, kind="ExternalInput")
        deltat = nc.dram_tensor("deltat", [128, PPC * NT], F32, kind="ExternalInput")
        out = nc.dram_tensor("out", [PPC, L, D], F32, kind="ExternalOutput")
        with tile.TileContext(nc) as tc:
            _emit(tc, qt.ap(), kt.ap(), v.ap(), tau4.ap(), deltat.ap(), out.ap())
        if CFG["SELF_LOAD"]:
            _compile_no_ldw_split(nc)
        else:
            nc.compile()
        _NC_CACHE["nc"] = nc
    return _NC_CACHE["nc"]


def _host_prep(queries, keys, values, tau, delta):
    """Shard + lay out full inputs into 8 per-core input maps."""
    queries = np.asarray(queries, np.float32)
    keys = np.asarray(keys, np.float32)
    values = np.asarray(values, np.float32)
    qT = np.ascontiguousarray(queries.transpose(0, 2, 3, 1)).reshape(PAIRS, E, L)
    kT = np.ascontiguousarray(keys.transpose(0, 2, 3, 1)).reshape(PAIRS, E, L)
    vv = np.ascontiguousarray(values.transpose(0, 2, 1, 3)).reshape(PAIRS, L, D)
    tau_flat = np.asarray(tau, np.float32).reshape(B)
    # delta^T per batch: [128, NT] where column c = delta[b, 128c:128c+128]
    dT = np.ascontiguousarray(
        np.asarray(delta, np.float32).reshape(B, NT, 128).transpose(0, 2, 1)
    )
    in_maps = []
    for m in range(NCORES):
        gs = range(PPC * m, PPC * (m + 1))
        bidx = [g // H for g in gs]
        in_maps.append(
            {
                "qt": np.ascontiguousarray(qT[PPC * m : PPC * (m + 1)]),
                "kt": np.ascontiguousarray(kT[PPC * m : PPC * (m + 1)]),
                "v": np.ascontiguousarray(vv[PPC * m : PPC * (m + 1)]),
                "tau4": tau_flat[bidx].reshape(1, PPC).copy(),
                "deltat": np.concatenate([dT[b] for b in bidx], axis=1),
            }
        )
    return in_maps


def _host_gather(per_core_outs):
    full = np.stack(per_core_outs).reshape(B, H, L, D)
    return np.ascontiguousarray(full.transpose(0, 2, 1, 3))


def kernel(queries, keys, values, tau, delta, **_):
    nc = _get_nc()
    in_maps = _host_prep(queries, keys, values, tau, delta)
    res = run_bass_kernel_spmd(nc, in_maps, list(range(NCORES)))
    return _host_gather([res.results[m]["out"] for m in range(NCORES)])



# revision 33
# speedup vs baseline: 1.0008x; 1.0008x over previous
"""DSAttention (de-stationary causal attention) Trainium2 Bass kernel.

Problem: B=4, L=S=2048, H=8, E=D=64, fp32.
  scores = (Q K^T) * tau_b + delta_b[s]      [B,H,L,S]
  A      = softmax(0.125 * scores) causal-masked
  out    = A V                               [B,L,H,D]

Sharding: the 32 (b,h) pairs are split 4-per-core across 8 NeuronCores
(data + head parallel). Each core runs an identical SPMD Bass program on
its own 4 pairs; no collectives are needed.

Per-core algorithm (two PAIRED windows, one per pair-pair):
  - Scores are computed TRANSPOSED, S^T[s, l], one 128-row s-chunk at a
    time. The two pairs of a pair-pair share ONE [128, L] k^T stationary
    tile (rows 0:64 = pair A, 64:128 = pair B); each pair's q^T moving
    tile is [128, L] with the OTHER pair's partition half ZEROED, so a
    full K=128 matmul computes that pair's scores exactly. K=128 matters
    beyond sharing one ldweights: the PE_HAM clock governor does NOT
    count K<=64 matmuls as activity (verified on HW - K=64/K=1 streams
    never lift the PE out of its 1.2GHz cold state), while this layout
    latches 2.4GHz right at the first QK burst and holds it.
  - q/k/v are loaded via gpsimd-initiated CASTING DMAs (fp32 DRAM ->
    bf16 SBUF in flight). The single SWDGE ring sustains ~130GB/s and
    carries ~6MB, so it is nearly co-critical: pair-pair 0 loads the
    tail half first (its chunks run high-c-first, MIX0), and the next
    window's q/k prefetch is emitted early in the current window.
    (Splitting the feed onto the sync HWDGE ring was tried twice and is
    SLOWER - that ring is no faster and the extra casts/staging stall.)
  - exp of the scores is split between two engines by a greedy
    makespan assignment with per-engine time models (ACT (w+420)/1.2,
    DVE (w*1.042+276)*DVE_LOAD) and an alternation preference (the
    3-tile psum ring buffers only ~1 piece, so same-engine streaks
    stall the other engine): ACT runs exact exp(scale*qk + bias) with
    fused per-partition scale=0.125*tau / bias=0.125*delta^T columns;
    DVE runs a Schraudolph fast-exp producing bf16 BITS via one
    tensor_scalar: int16(qk*c1 + c2), bitcast to bf16 (~2% rms error on
    its share). GpSimd cannot read PSUM and DMA has no PSUM route, so
    ACT+DVE are the ONLY possible score consumers - their combined
    throughput is the kernel's floor.
  - The causal mask only affects the diagonal 128x128 block of each
    chunk; it is applied AFTER exp as a triangle zeroing via
    gpsimd.affine_select (keeps the hot ACT/DVE engines free).
  - AV uses A^T chunks (bf16) as the 128x128 stationary operand and an
    augmented moving operand [v | 1] of 65 columns, so column 64 of the
    PSUM accumulator is the softmax denominator (ldweights pipelines
    into the background weight buffer; AV sustains ~33ns/step warm).
    Output tiles come out in natural [l, d] layout; DVE computes
    reciprocal + scale, sync DMAs store.
  - SCHEDULE: window 0 = QK(pp0) exp-paced; window 1 = QK(pp1) with
    pair-pair 0's deferred AV pumped between score pieces in small
    budget-paced units (coarse AV bursts starve the exp engines);
    window-1's own AV fires inline as chunks complete, with a pump
    floor late in the window so the no-exp-overlap tail stays short.
    K=128 garbage-weight filler matmuls ramp the PE clock during the
    input DMA dead time (the fillers read a raw uninitialized SBUF
    tensor so they depend on nothing after the engine barrier).
"""

import numpy as np

try:
    import concourse.bass as bass
except ImportError:  # toolchain not on default path
    import sys

    sys.path.insert(0, "/opt/trn_rl_repo")
    import concourse.bass as bass

import concourse.mybir as mybir
import concourse.tile as tile
from concourse import bacc
from concourse.bass_utils import run_bass_kernel_spmd

B, L, H, E, D = 4, 2048, 8, 64, 64
NCORES = 8
PAIRS = B * H            # 32 (b,h) pairs
PPC = PAIRS // NCORES    # 4 pairs per core
NT = L // 128            # 16 s-chunks / l-tiles per pair
MMW = 512                # max moving width per fp32-out matmul (1 PSUM bank)

F32 = mybir.dt.float32
BF16 = mybir.dt.bfloat16

import os as _os

CFG = {
    "PIECE": int(_os.environ.get("DSATT_PIECE", "1024")),
    "PS_BUFS": int(_os.environ.get("DSATT_PS_BUFS", "3")),
    "JBATCH": int(_os.environ.get("DSATT_JBATCH", "7")),
    "QSPLIT": int(_os.environ.get("DSATT_QSPLIT", "2")),  # first pair-pair
    "QSPLIT2": int(_os.environ.get("DSATT_QSPLIT2", "2")),  # later pair-pairs
    "TAILSPLIT": int(_os.environ.get("DSATT_TAILSPLIT", "1")),
    "BCAST_NORM": int(_os.environ.get("DSATT_BCAST_NORM", "1")),
    # legalize multi-wait matmuls via event semaphores instead of bacc's
    # move_matmul_waits_to_ldweights pass (HW-validated variant)
    "SELF_LOAD": int(_os.environ.get("DSATT_SELF_LOAD", "1")),
    # fraction of exp work offloaded from ACT to DVE / GpSimd using the
    # Schraudolph bf16-bits fast-exp (int16(x*c1+c2) bitcast to bf16)
    # NOTE: GpSimd cannot read PSUM, so only ACT and DVE can consume QK
    # scores; EXP_POOL must stay 0 unless scores are staged to SBUF first.
    "EXP_DVE": float(_os.environ.get("DSATT_EXP_DVE", "0.44")),
    "EXP_POOL": float(_os.environ.get("DSATT_EXP_POOL", "0.0")),
    # defer each pair's AV into the next pair's QK window (0 = fire AV
    # batches inline as soon as their chunks are ready, as before)
    "AV_DEFER": int(_os.environ.get("DSATT_AV_DEFER", "1")),
    # deferred-AV pump ratio: AV cycles emitted per QK cycle in the next
    # pair's window
    "AV_RATIO": float(_os.environ.get("DSATT_AV_RATIO", "1.0")),
    # filler matmuls at kernel start to ramp the PE clock during input DMA
    "WARM": int(_os.environ.get("DSATT_WARM", "16")),
    # window-0 pacing: filler matmul columns emitted after each pair-0 QK
    # piece (pair 0 has no deferred AV to interleave, so raw QK outruns
    # the exp consumers, stalls, and resets the PE pstate ramp)
    "W0PACE": int(_os.environ.get("DSATT_W0PACE", "0")),
    # pair-0 mixed chunk order: this many high chunks first, then 0..N.
    # 16 = pure high-first (verified schedule; MIX0=4 raised the error to
    # 3.95e-2 by clustering fast-exp pieces onto low-l rows)
    "MIX0": int(_os.environ.get("DSATT_MIX0", "8")),
    # order-independent exp engine assignment (enables safe reordering)
    "EXP_HASH": int(_os.environ.get("DSATT_EXP_HASH", "0")),
    "WARM_COLS": int(_os.environ.get("DSATT_WARM_COLS", "512")),
    # run QK matmuls in float32r straight from the fp32 q/k tiles (no
    # casts). BROKEN on HW: walrus fp32r weights via split ldweights give
    # wrong results (rel err ~1). Keep 0.
    "QK_F32R": int(_os.environ.get("DSATT_QK_F32R", "0")),
    # move q/k bf16 casts for pair-pairs > 0 to GpSimd (SBUF->SBUF ok)
    "CAST_POOL": int(_os.environ.get("DSATT_CAST_POOL", "0")),
    # move v bf16 casts to GpSimd
    "VCAST_POOL": int(_os.environ.get("DSATT_VCAST_POOL", "0")),
    # fp32->bf16 casting DMAs (gpsimd-initiated) for q/k and v loads:
    # 2 = all pair-pairs, 1 = pair-pairs > 0 only, 0 = DVE casts.
    "QKDMA_CAST": int(_os.environ.get("DSATT_QKDMA_CAST", "2")),
    "VDMA_CAST": int(_os.environ.get("DSATT_VDMA_CAST", "1")),
    # split the input feed across DMA rings: qB + v via sync(HWDGE) fp32
    # + gpsimd SBUF casts instead of gpsimd casting DMAs
    "SPLIT_FEED": int(_os.environ.get("DSATT_SPLIT_FEED", "0")),
}

# fast-exp: bf16 bits of exp(v) ~= round(v * A16 + B16) taken as int16
FE_A16 = 128.0 / float(np.log(2.0))  # 184.6650
FE_B16 = float(_os.environ.get("DSATT_FE_B16", "16250.4"))  # 16256 - C*


def _compile_no_ldw_split(nc):
    """bacc.Bacc.compile() minus move_matmul_waits_to_ldweights: keeps
    matmuls self-loading; generate_event_semaphores legalizes waits."""
    from concourse import inst_simplify

    nc.insert_bir_kernel_barrier_sem_inc()
    nc.generate_event_semaphores()
    nc.remove_dead_instructions_after_branch()
    nc.validate_blocks()
    nc.dce_regs()
    nc.thread_jumps()
    nc.remove_dead_blocks()
    nc.remove_dead_allocations()
    nc.verify_switch_hints()
    nc.alloc_regs()
    inst_simplify.simplify(nc)
    nc.fuse_regops()
    nc.fuse_blocks()
    nc.replace_nops_with_events()
    for engine in nc.engines:
        nc.fuse_nops(engine)
    nc.remove_dead_nops()
    nc.remove_dangling_data()
    nc.generate_event_semaphores()
    nc.insert_library_loads()
    nc.insert_act_table_loads()
    nc.insert_hostgen_rebases()
    nc.codegen_inst_isa_subclasses()


def _emit(tc, qt, kt, v, tau4, deltat, out):
    nc = tc.nc
    Exp = mybir.ActivationFunctionType.Exp
    from contextlib import ExitStack

    PIECE = CFG["PIECE"]
    JBATCH = CFG["JBATCH"]
    QSPLIT = CFG["QSPLIT"]

    ctx = ExitStack()
    const = ctx.enter_context(tc.tile_pool(name="const", bufs=1))
    qk_pool = ctx.enter_context(tc.tile_pool(name="qk", bufs=2))
    qkb_pool = ctx.enter_context(tc.tile_pool(name="qkb", bufs=2))
    v_pool = ctx.enter_context(tc.tile_pool(name="vp", bufs=3))
    vb_pool = ctx.enter_context(tc.tile_pool(name="vbp", bufs=3))
    at_pool = ctx.enter_context(tc.tile_pool(name="atp", bufs=2))
    ob_pool = ctx.enter_context(tc.tile_pool(name="obp", bufs=3))
    ps_pool = ctx.enter_context(tc.tile_pool(name="psp", bufs=CFG["PS_BUFS"], space="PSUM"))
    po_pool = ctx.enter_context(tc.tile_pool(name="pop", bufs=2, space="PSUM"))

    # ---- PE clock pre-warm: filler matmuls while input DMA is in flight.
    # The TRN2 tensor engine runs at the 1.2GHz mid pstate until it has been
    # continuously busy for ~3us; these ramp it so real QK work starts fast.
    # The filler operand is a RAW (non-tile) SBUF tensor read uninitialized:
    # any init (memset/DMA) would chain the first filler behind another
    # engine's startup, delaying the HAM warm ramp. Garbage values are
    # harmless - the filler PSUM banks are fully overwritten (start=True)
    # by every real matmul before being read.
    # K=128 full-array fillers: low-utilization matmuls (K=1) do NOT
    # register as busy to the PE_HAM activity monitor (verified on HW: a
    # 4.5us continuous K=1 filler block left the clock at 1.2GHz), so the
    # warm-up must exercise all 128 rows.
    cw_h = ctx.enter_context(nc.sbuf_tensor("warm_cw", [128, 512], BF16))
    cw = cw_h.ap()
    if CFG["WARM"]:
        wc = min(CFG["WARM_COLS"], CFG["PIECE"])
        pw = ps_pool.tile([128, CFG["PIECE"]], F32, tag="ps", name="warm")
        for _ in range(CFG["WARM"]):
            nc.tensor.matmul(pw[:, 0:wc], lhsT=cw[:, 0:128], rhs=cw[:, 0:wc], start=True, stop=True)
    # preload the ACT exp spline tables (~1.3us) off the critical path: the
    # compiler inserts ACT_TABLE_LOAD before the first ACTIVATE in program
    # order, which would otherwise sit right before the first real exp
    warm_act = const.tile([1, 8], BF16)
    nc.scalar.activation(warm_act[:], cw[0:1, 0:8], Exp)

    # ---- q/k loads for a pair-pair ---------------------------------------
    # k is one SHARED [128, L] stationary tile (rows 0:64 = pair A's k^T,
    # 64:128 = pair B's). Each pair's q is a SEPARATE [128, L] moving tile
    # with the OTHER pair's partition half zeroed, so a full K=128 matmul
    # against the shared k computes that pair's scores exactly (the other
    # half contributes 0). K=128 matmuls register as PE_HAM activity
    # (K<=64 ones do NOT - verified on HW), keeping the 2.4GHz clock, and
    # one ldweights serves both pairs.
    def emit_qk_loads(pp):
        kb = qkb_pool.tile([128, L], BF16, tag="kb")
        qbA = qkb_pool.tile([128, L], BF16, tag="qa")
        qbB = qkb_pool.tile([128, L], BF16, tag="qb")
        # zero the unused partition half of each q tile (DVE is idle at
        # startup; prefetched window-1 tiles are zeroed on gpsimd)
        mset = nc.vector.memset if pp == 0 else nc.gpsimd.memset
        mset(qbA[64:128, :], 0.0)
        mset(qbB[0:64, :], 0.0)
        kdram = kt[2 * pp : 2 * pp + 2].rearrange("a e l -> (a e) l")
        nsplit = QSPLIT if pp == 0 else CFG["QSPLIT2"]
        w = L // nsplit
        # pair-pair 0 runs its chunks high-c-first (they need only the tail
        # columns of q/k), so load pieces tail-first to start PE early
        order = range(nsplit - 1, -1, -1) if pp == 0 else range(nsplit)
        for i in order:
            s = slice(w * i, w * (i + 1))
            nc.gpsimd.dma_start(kb[:, s], kdram[:, s])
            nc.gpsimd.dma_start(qbA[0:64, s], qt[2 * pp][:, s])
            if CFG["SPLIT_FEED"]:
                qf = qk_pool.tile([128, w], F32, tag="qf", name=f"qf_{pp}_{i}")
                nc.sync.dma_start(qf[64:128, :], qt[2 * pp + 1][:, s])
                nc.gpsimd.tensor_copy(qbB[64:128, s], qf[64:128, :])
            else:
                nc.gpsimd.dma_start(qbB[64:128, s], qt[2 * pp + 1][:, s])
        return kb, qbA, qbB

    # setup DMAs go first on the sync queue so the ptau matmul (PE queue,
    # right after the warm fillers) isn't blocked behind bulk q/k loads
    c0125 = const.tile([1, 128], F32)
    nc.vector.memset(c0125[:], 0.125)
    tau_sb = const.tile([1, PPC], F32)
    nc.sync.dma_start(tau_sb[:], tau4[:])
    dts = const.tile([128, PPC * NT], F32)
    nc.sync.dma_start(dts[:], deltat[:])

    # pair-pair 0's loads go next so the first QK matmul isn't queued
    # behind the tau/delta setup chain on DVE
    _cur_qk = emit_qk_loads(0)

    # ---- one-time setup -------------------------------------------------
    # Broadcast 0.125*tau[p] to all 128 partitions via a K=1 matmul.
    ptau = po_pool.tile([128, JBATCH * (D + 1)], F32, tag="po", name="ptau")
    nc.tensor.matmul(ptau[:, 0:PPC], lhsT=c0125[:], rhs=tau_sb[:], start=True, stop=True)
    tau_cols = const.tile([128, PPC], F32)
    nc.vector.tensor_copy(tau_cols[:], ptau[:, 0:PPC])

    # bias columns: 0.125 * delta^T  ([128, PPC*NT], column p*NT+c)
    bias_all = const.tile([128, PPC * NT], F32)
    nc.vector.tensor_scalar_mul(bias_all[:], dts[:], 0.125)

    # fast-exp scale/bias columns: int16 bf16-bits = qk*(A16*0.125*tau) +
    # (A16*0.125*delta + B16)
    tsc1 = const.tile([128, PPC], F32)
    nc.vector.tensor_scalar_mul(tsc1[:], tau_cols[:], FE_A16)
    fb = const.tile([128, PPC * NT], F32)
    nc.vector.tensor_scalar(
        fb[:], bias_all[:], FE_A16, FE_B16,
        mybir.AluOpType.mult, mybir.AluOpType.add,
    )

    # greedy makespan assignment of exp pieces using per-engine TIME
    # models (ACT: (w+420)/1.2 ns; DVE: w*1.042+276 ns, inflated by
    # DVE_LOAD for its reciprocal/normalize side work), with a mild
    # preference to alternate engines (the 3-deep psum ring can only
    # buffer ~1 piece, so same-engine streaks stall the other engine)
    exp_done = {"act": 0.0, "dve": 0.0}
    _last_eng = [None]
    _dve_load = float(_os.environ.get("DSATT_DVE_LOAD", "1.12"))

    def next_exp_engine(w, pair, c, pidx):
        t = {"act": (w + 420) / 1.2, "dve": (w * 1.042 + 276) * _dve_load}
        cand = sorted(("act", "dve"), key=lambda e: exp_done[e] + t[e])
        e = cand[0]
        if e == _last_eng[0] and (
            exp_done[cand[1]] + t[cand[1]] - (exp_done[e] + t[e]) < 500
        ):
            e = cand[1]
        exp_done[e] += t[e]
        _last_eng[0] = e
        return e

    I16 = mybir.dt.int16

    def emit_exp(atc_slice, pst_slice, pair, c, w, pidx):
        eng = next_exp_engine(w, pair, c, pidx)
        if eng == "act":
            nc.scalar.activation(
                atc_slice, pst_slice, Exp,
                bias=bias_all[:, NT * pair + c : NT * pair + c + 1],
                scale=tau_cols[:, pair : pair + 1],
            )
        else:
            e = nc.vector if eng == "dve" else nc.gpsimd
            e.tensor_scalar(
                atc_slice.bitcast(I16), pst_slice,
                tsc1[:, pair : pair + 1],
                fb[:, NT * pair + c : NT * pair + c + 1],
                mybir.AluOpType.mult, mybir.AluOpType.add,
            )

    # ---- deferred-AV queue: each pair's AV matmuls are emitted during
    # the NEXT pair's QK window, interleaved ~1:1 in PE cycles. The PE
    # then alternates QK pieces (which need exp consumers) with AV steps
    # (which don't), halving the burst demand on ACT/DVE and avoiding
    # the PSUM-full stalls that knock the PE down to its 1.2GHz pstate.
    av_queue = []  # FIFO of (est_pe_cycles, emit_thunk, is_batch_start)

    def pump_av(budget):
        while av_queue and budget > 0:
            cyc, fn, _ = av_queue.pop(0)
            fn()
            budget -= cyc

    def build_av_units(pair, ats, vb3, j0, nj):
        units = []
        box = {}

        def mmj(jl):
            if jl == 0:
                box["pot"] = po_pool.tile(
                    [128, JBATCH * (D + 1)], F32, tag="po", name=f"po_{pair}_{j0}"
                )
            pot = box["pot"]
            j = j0 + jl
            for cc in range(j + 1):
                nc.tensor.matmul(
                    pot[:, 65 * jl : 65 * jl + 65],
                    lhsT=ats[cc][:, 128 * (j - cc) : 128 * (j - cc) + 128],
                    rhs=vb3[:, cc, :],
                    start=(cc == 0),
                    stop=(cc == j),
                )

        for jl in range(nj):
            units.append(((j0 + jl + 1) * 128, (lambda jl=jl: mmj(jl)), jl == 0))

        def fin():
            pot3 = box["pot"].rearrange("q (jl x) -> q jl x", x=D + 1)
            rec = ob_pool.tile([128, JBATCH], F32, tag="rec", name=f"rec_{pair}_{j0}")
            nc.vector.reciprocal(rec[:, 0:nj], pot3[:, 0:nj, D])
            ob = ob_pool.tile([128, JBATCH * D], F32, tag="ob", name=f"ob_{pair}_{j0}")
            ob3 = ob.rearrange("q (jl d) -> q jl d", d=D)
            if CFG.get("BCAST_NORM", 1):
                in0b, in1b = bass.broadcast_tensor_aps(
                    pot3[:, 0:nj, 0:D], rec[:, 0:nj].unsqueeze(2)
                )
                nc.vector.tensor_tensor(
                    out=ob3[:, 0:nj, :], in0=in0b, in1=in1b,
                    op=mybir.AluOpType.mult,
                )
            else:
                for jl in range(nj):
                    nc.vector.tensor_scalar_mul(
                        ob[:, D * jl : D * jl + D],
                        pot3[:, jl, 0:D],
                        rec[:, jl : jl + 1],
                    )
            nc.sync.dma_start(
                out[pair, 128 * j0 : 128 * (j0 + nj), :].rearrange(
                    "(jl q) d -> q jl d", q=128
                ),
                ob.rearrange("q (jl d) -> q jl d", d=D)[:, 0:nj, :],
            )

        units.append((300, fin, False))
        return units

    # ---- paired windows: the two pairs of a pair-pair (row halves h0 /
    # h64 of the PE array) run their QK matmuls ADJACENTLY, so the two
    # K=64 matmuls execute CONCURRENTLY in disjoint row groups (~2x QK
    # throughput). Window 0 = QK(pp0) [exp-paced; PE has headroom],
    # window 1 = QK(pp1) + deferred AV(pp0); tail = leftover AV(pp1),
    # partially pulled inline as its chunks complete.
    def emit_v_load(pair):
        vb = vb_pool.tile([128, NT * (D + 1)], BF16, tag="vb")
        vb3 = vb.rearrange("q (c x) -> q c x", x=D + 1)
        nc.gpsimd.memset(vb3[:, :, D : D + 1], 1.0)
        if CFG["SPLIT_FEED"]:
            # sync (HWDGE) fp32 load + gpsimd cast: keeps the v traffic
            # off the oversubscribed gpsimd SWDGE DMA ring
            vf = v_pool.tile([128, NT * D], F32, tag="vf")
            nc.sync.dma_start(
                vf.rearrange("q (c d) -> q c d", d=D),
                v[pair].rearrange("(c q) d -> q c d", q=128),
            )
            nc.gpsimd.tensor_copy(
                vb3[:, :, 0:D], vf.rearrange("q (c d) -> q c d", d=D)
            )
        elif CFG["VDMA_CAST"]:
            nc.gpsimd.dma_start(
                vb3[:, :, 0:D],
                v[pair].rearrange("(c q) d -> q c d", q=128),
            )
        else:
            vf = v_pool.tile([128, NT * D], F32, tag="vf")
            nc.sync.dma_start(
                vf.rearrange("q (c d) -> q c d", d=D),
                v[pair].rearrange("(c q) d -> q c d", q=128),
            )
            nc.vector.tensor_copy(
                vb3[:, :, 0:D], vf.rearrange("q (c d) -> q c d", d=D)
            )
        return vb3

    def make_batches(pair):
        if CFG["TAILSPLIT"] and pair >= PPC - 2:
            # last window: close the big batches a couple of chunks early
            # and keep the final ones tiny, so nearly all AV is pumpable
            # before the last exp lands and the drain tail stays short
            return [(0, 7), (7, 6), (13, 2), (15, 1)]
        batches = []
        j0_ = 0
        while j0_ < NT:
            nj_ = min(JBATCH, NT - j0_)
            batches.append((j0_, nj_))
            j0_ += nj_
        return batches

    NPP = PPC // 2
    _prefetched_qk = None
    for pp in range(NPP):
        pA, pB = 2 * pp, 2 * pp + 1
        if pp > 0:
            _cur_qk = _prefetched_qk if _prefetched_qk else emit_qk_loads(pp)
        kb, qbA, qbB = _cur_qk
        qtiles = {pA: qbA, pB: qbB}
        vb3s = {pA: emit_v_load(pA), pB: emit_v_load(pB)}
        batches = {pA: make_batches(pA), pB: make_batches(pB)}
        ats = {pA: [None] * NT, pB: [None] * NT}
        done = {pA: set(), pB: set()}
        fired = {pA: set(), pB: set()}
        last_pp = pp == NPP - 1
        ratio = CFG["AV_RATIO"] if CFG["AV_DEFER"] else float(1 << 20)
        if pp == 0:
            # high chunks first (need only tail q/k DMA pieces), then
            # ascending so low chunks complete early for AV readiness
            m0 = CFG["MIX0"]
            chunk_iter = list(range(NT - 1, NT - 1 - m0, -1)) + list(range(NT - m0))
        else:
            chunk_iter = range(NT)
        _next_qk = None
        for ci, c in enumerate(chunk_iter):
            # prefetch the next pair-pair's q/k early in this window so
            # its ~2MB casting DMA overlaps this window's exp-paced work
            if ci == 2 and pp + 1 < NPP:
                _next_qk = emit_qk_loads(pp + 1)
            ext = L - 128 * c
            for p in (pA, pB):
                ats[p][c] = at_pool.tile(
                    [128, ext], BF16, tag=f"at{c}{p % 2}", name=f"at{c}_{p}"
                )
            l0 = 128 * c
            while l0 < L:
                w = min(PIECE, L - l0)
                pst = {
                    p: ps_pool.tile([128, PIECE], F32, tag="ps", name=f"ps_{p}_{c}_{l0}")
                    for p in (pA, pB)
                }
                # full K=128 matmuls against the shared k stationary; the
                # other pair's zeroed q half contributes exact zeros.
                # 512-aligned subs (PSUM bank rule); pieces alternate pairs
                # so the exp deficit scheduler alternates ACT/DVE.
                for p in (pA, pB):
                    for s0 in range(0, w, MMW):
                        sw = min(MMW, w - s0)
                        nc.tensor.matmul(
                            pst[p][:, s0 : s0 + sw],
                            lhsT=kb[:, 128 * c : 128 * (c + 1)],
                            rhs=qtiles[p][:, l0 + s0 : l0 + s0 + sw],
                            start=True,
                            stop=True,
                        )
                off = l0 - 128 * c
                for p in (pA, pB):
                    emit_exp(
                        ats[p][c][:, off : off + w], pst[p][:, 0:w], p, c, w, off // PIECE
                    )
                if not CFG["AV_DEFER"]:
                    pump_av(1 << 30)
                elif ci < NT - 6:
                    pump_av(int(2 * w * ratio))
                else:
                    # late-window: QK pieces shrink while the AV backlog
                    # peaks - pump with an absolute floor so the kernel
                    # tail (AV with no exp overlap) stays short
                    pump_av(max(int(2 * w * 3.0), 16000 if ci >= NT - 3 else 8000))
                l0 += w
            for p in (pA, pB):
                # zero the strictly-lower triangle (s > l) of the diag block
                nc.gpsimd.affine_select(
                    out=ats[p][c][:, 0:128],
                    in_=ats[p][c][:, 0:128],
                    compare_op=mybir.AluOpType.is_ge,
                    fill=0.0,
                    base=0,
                    pattern=[[1, 128]],
                    channel_multiplier=-1,
                )
                done[p].add(c)
                # queue ready AV batches inline (their leftovers carry
                # into the next window's pump)
                for j0, nj in batches[p]:
                    need = j0 + nj - 1
                    if (j0, nj) in fired[p] or not all(
                        cc in done[p] for cc in range(need + 1)
                    ):
                        continue
                    fired[p].add((j0, nj))
                    av_queue.extend(build_av_units(p, ats[p], vb3s[p], j0, nj))
        # window end: queue any remaining batches for the next window
        _prefetched_qk = _next_qk
        for p in (pA, pB):
            for j0, nj in batches[p]:
                if (j0, nj) not in fired[p]:
                    fired[p].add((j0, nj))
                    av_queue.extend(build_av_units(p, ats[p], vb3s[p], j0, nj))
    pump_av(1 << 30)
    ctx.close()


_NC_CACHE = {}


def _get_nc():
    if "nc" not in _NC_CACHE:
        nc = bacc.Bacc("TRN2", target_bir_lowering=False, debug=False)
        qt = nc.dram_tensor("qt", [PPC, E, L], F32, kind="ExternalInput")
        kt = nc.dram_tensor("kt", [PPC, E, L], F32, kind="ExternalInput")
        v = nc.dram_tensor("v", [PPC, L, D], F32, kind="ExternalInput")
        tau4 = nc.dram_tensor("tau4", [1, PPC], F32, kind="ExternalInput")
        deltat = nc.dram_tensor("deltat", [128, PPC * NT], F32, kind="ExternalInput")
        out = nc.dram_tensor("out", [PPC, L, D], F32, kind="ExternalOutput")
        with tile.TileContext(nc) as tc:
            _emit(tc, qt.ap(), kt.ap(), v.ap(), tau4.ap(), deltat.ap(), out.ap())
        if CFG["SELF_LOAD"]:
            _compile_no_ldw_split(nc)
        else:
            nc.compile()
        _NC_CACHE["nc"] = nc
    return _NC_CACHE["nc"]


def _host_prep(queries, keys, values, tau, delta):
    """Shard + lay out full inputs into 8 per-core input maps."""
    queries = np.asarray(queries, np.float32)
    keys = np.asarray(keys, np.float32)
    values = np.asarray(values, np.float32)
    qT = np.ascontiguousarray(queries.transpose(0, 2, 3, 1)).reshape(PAIRS, E, L)
    kT = np.ascontiguousarray(keys.transpose(0, 2, 3, 1)).reshape(PAIRS, E, L)
    vv = np.ascontiguousarray(values.transpose(0, 2, 1, 3)).reshape(PAIRS, L, D)
    tau_flat = np.asarray(tau, np.float32).reshape(B)
    # delta^T per batch: [128, NT] where column c = delta[b, 128c:128c+128]
    dT = np.ascontiguousarray(
        np.asarray(delta, np.float32).reshape(B, NT, 128).transpose(0, 2, 1)
    )
    in_maps = []
    for m in range(NCORES):
        gs = range(PPC * m, PPC * (m + 1))
        bidx = [g // H for g in gs]
        in_maps.append(
            {
                "qt": np.ascontiguousarray(qT[PPC * m : PPC * (m + 1)]),
                "kt": np.ascontiguousarray(kT[PPC * m : PPC * (m + 1)]),
                "v": np.ascontiguousarray(vv[PPC * m : PPC * (m + 1)]),
                "tau4": tau_flat[bidx].reshape(1, PPC).copy(),
                "deltat": np.concatenate([dT[b] for b in bidx], axis=1),
            }
        )
    return in_maps


def _host_gather(per_core_outs):
    full = np.stack(per_core_outs).reshape(B, H, L, D)
    return np.ascontiguousarray(full.transpose(0, 2, 1, 3))


def kernel(queries, keys, values, tau, delta, **_):
    nc = _get_nc()
    in_maps = _host_prep(queries, keys, values, tau, delta)
    res = run_bass_kernel_spmd(nc, in_maps, list(range(NCORES)))
    return _host_gather([res.results[m]["out"] for m in range(NCORES)])

